# revision 88
# baseline (speedup 1.0000x reference)
"""Trainium2 Bass kernel: causal sliding-window GQA self-attention.

Problem: B=2, T=2048, C=2048, 16 q-heads / 4 kv-heads, head_dim=128,
RoPE, sliding window 512, projections Wq/Wk/Wv/Wo.

Sharding: 8 cores = DP(batch=2) x TP(head-groups=4).  Core c handles
batch c//4 and q-heads [4*(c%4), 4*(c%4)+4) (one kv head c%4).  Each
core computes a partial output contribution [T, C]; the host sums the
4 head-group partials per batch and divides by the 64^2 weight
pre-scale.

Per-core kernel (v2 — rebalanced against the TimelineSim cost model):
  - V projection and Wo run 3-term fp8-e4m3 DoubleRow (hi/lo split).
    Q and K run 2-term "A-variant": (xhi+xlo) @ Whi, pairing each cb's
    (hi, lo) x-chunks against the SAME weight chunk via a stride-0
    broadcast AP — full x precision, no Wqlo/Wklo inputs at all (-41k
    PE cycles, -4.3us serial DMA).  V cannot drop its third term: its
    error passes through the softmax average unsuppressed (relative
    error is conserved through a weighted mean).
  - All scale constants (1/sqrt(hd), 1/WSC^2) fold into the exp's
    immediate `scale` operand, so ONE raw cos/sin bf16 table pair
    serves both Q and K rope (halves the table DMA on the serial DMA
    queue, which co-limits phase A).
  - Rope eviction: PSUM->bf16 copy on ScalarE, then 3 bf16 all-SBUF
    DVE muls (2x DVE mode) + GpSimd add — 2.0x less DVE time than
    f32 muls straight out of PSUM.
  - Attention per 128-query block: score pairs in [128,1024] PSUM
    tiles; the (lead, diag) edge blocks share one tile so ONE DVE
    [128,1024] multiply applies both band-edge masks.  exp on ScalarE
    (no max subtraction), denominator tree 3 DVE adds + 1 GpSimd add +
    GpSimd partition_all_reduce, reciprocal/normalize/fp8-y-pair DVE.
  - Wo PSUM evictions: 2 DVE + 2 ScalarE per block, emitted after the
    denominator-chain ops so they never delay the exps that gate the
    score-PSUM ring (balanced: DVE/ScalarE/Pool all just under PE's
    4.7us/block).
  - Attention is software-pipelined 2 deep: scores(qb_k) emits with
    attn_rest(qb_{k-2}), hiding the ~6us exp/mask/denominator chain;
    phase B runs qb15 first so its Wo drain overlaps qb3..14.
  - x streams hi/lo-interleaved tb-major in [128,4,512] pieces (512-byte
    DMA runs dodge the sub-512B 2x latency penalty on the serial DMA
    queue); tiny first transfers (2-chunk wvhi/x slices) cut the
    first-DMA pipeline-latency gate at program start.
"""

import os
import sys

for _p in ("/opt/trn_rl_repo", "/root/.axon_site/_ro/trn_rl_repo"):
    if os.path.isdir(_p) and _p not in sys.path:
        sys.path.append(_p)

import numpy as np
import ml_dtypes

BF16 = ml_dtypes.bfloat16
F8 = ml_dtypes.float8_e4m3fn

B, T, C = 2, 2048, 2048
H, KVH, HD = 16, 4, 128
WIN = 512
ROPE_BASE = 10000.0
NCORES = 8
TPG = 4           # tensor-parallel group count (head groups)
HPG = H // TPG    # q-heads per core
SCALE = 1.0 / float(np.sqrt(np.float32(HD)))
NWINB = WIN // 128 + 1   # 5 key blocks cover the 640-wide window
NCB = C // 128           # contraction row-blocks for projections
WSC = 64.0               # host weight pre-scale (fp8 subnormal dodge)
EXP_SCALE = float(SCALE / (WSC * WSC))  # folded into the exp activation

_NC_CACHE = {}


def _rope_tables(t_len):
    # Match reference: angles computed in float32.  Raw tables (no folded
    # constants): cosT [128, T], sin_swap [128, T] with the rotate-half sign.
    inv = (1.0 / (np.float32(ROPE_BASE) ** (np.arange(0, HD, 2, dtype=np.float32) / np.float32(HD)))).astype(np.float32)
    ang = np.arange(t_len, dtype=np.float32)[None, :] * inv[:, None]   # [64, T]
    cosT = np.concatenate([np.cos(ang), np.cos(ang)], axis=0)          # [128, T]
    sinT = np.sin(ang)
    # Row-halves pre-swapped (sinB[d] = sin_swap[(d+64)%128]) so the kernel's
    # two half-muls read zb and the table at EQUAL base partitions (the bir
    # verifier rejects SBUF input pairs at different bases):
    #   t2[64:128] = zb[0:64]   * sinB[0:64]   (= +sin rows)
    #   t2[0:64]   = zb[64:128] * sinB[64:128] (= -sin rows)
    sinB = np.concatenate([sinT, -sinT], axis=0)                       # [128, T]
    return cosT.astype(np.float32), sinB.astype(np.float32)


def _mask_edge4():
    # maskE[c, 0:512]  = lead mask (key col c allowed for query r iff c >= r+1)
    # maskE[c, 512:1024] = diag mask (allowed iff c <= r)
    # each replicated x4 along the free dim for the 4-head-batched layout.
    r = np.arange(128)[None, :]
    c = np.arange(128)[:, None]
    lead = (c >= r + 1).astype(np.float32)          # [128, 128]
    diag = (c <= r).astype(np.float32)
    return np.concatenate([np.tile(lead, (1, HPG)), np.tile(diag, (1, HPG))], axis=1)


def _split_f8(a):
    """fp8 hi/lo pair: a ~ hi + lo with ~7-bit effective mantissa."""
    a32 = np.asarray(a, np.float32)
    hi = a32.astype(F8)
    lo = (a32 - hi.astype(np.float32)).astype(F8)
    return hi, lo


def build_nc(t_len=T):
    """Build + compile the per-core Bass module (SPMD, identical on all cores)."""
    import concourse.mybir as mybir
    import concourse.tile as tile
    from concourse import bacc
    from concourse import bass_isa

    dt = mybir.dt
    DRow = mybir.MatmulPerfMode.DoubleRow
    NQB = t_len // 128        # query/key blocks
    NTB = t_len // 512        # 512-wide t-blocks for projections

    nc = bacc.Bacc("TRN2", target_bir_lowering=False, debug=False, num_devices=NCORES)

    def din(name, shape, d=dt.float8e4):
        return nc.dram_tensor(name, shape, d, kind="ExternalInput").ap()

    xhi_d = din("xhi", [C, t_len])
    xlo_d = din("xlo", [C, t_len])
    wqhi_d = din("wqhi", [C, HPG * HD])
    wkhi_d = din("wkhi", [C, HD])
    wvhi_d = din("wvhi", [C, HD])
    wvlo_d = din("wvlo", [C, HD])
    wohi_d = din("wohi", [HPG * HD, C])
    wolo_d = din("wolo", [HPG * HD, C])
    cos_d = din("cosT", [HD, t_len], dt.bfloat16)
    sin_d = din("sinT", [HD, t_len], dt.bfloat16)
    maskE_d = din("maskE", [128, 2 * HPG * 128], dt.bfloat16)
    out_d = nc.dram_tensor("out", [t_len, C], dt.bfloat16, kind="ExternalOutput").ap()

    with tile.TileContext(nc) as tc:
        with tc.tile_pool(name="persist", bufs=1) as pp:
            f8 = dt.float8e4
            # x2: hi/lo chunk-interleaved: [p, cb(16), hilo(2), t]
            x2_sb = pp.tile([128, NCB * 2 * t_len], f8, tag="x2")
            wqhi_sb = pp.tile([128, NCB * HPG * HD], f8, tag="wqhi")
            wkhi_sb = pp.tile([128, NCB * HD], f8, tag="wkhi")
            wvhi_sb = pp.tile([128, NCB * HD], f8, tag="wvhi")
            wvlo_sb = pp.tile([128, NCB * HD], f8, tag="wvlo")
            wohi_sb = pp.tile([128, HPG * C], f8, tag="wohi")
            wolo_sb = pp.tile([128, HPG * C], f8, tag="wolo")
            cos_sb = pp.tile([128, t_len], dt.bfloat16, tag="cosT")
            sin_sb = pp.tile([128, t_len], dt.bfloat16, tag="sinT")
            maskE_sb = pp.tile([128, 2 * HPG * 128], dt.bfloat16, tag="maskE")
            QT4_sb = pp.tile([128, NQB * HPG * 128], dt.bfloat16, tag="QT4")
            KT_sb = pp.tile([128, t_len], dt.bfloat16, tag="KT")
            V_sb = pp.tile([128, t_len], dt.bfloat16, tag="V")

            def x4(hilo):
                # [p, cb, t] view of the hi or lo planes of x2
                return x2_sb[:].rearrange("p (c h t) -> p c h t", h=2, t=t_len)[:, :, hilo, :]

            def wv_(ts, m):
                return ts[:].rearrange("p (c m) -> p c m", m=m)

            # ---- DMA emission order (serial DMA queue = this order).
            # x streams per-(tb, hilo, 4-chunk-group) in [128,4,512] pieces
            # (512-byte runs dodge the sub-512B 2x DMA latency penalty);
            # projection matmuls chase the chunk stream, so PE starts after
            # wvhi + the first chunk group (~3.5us).
            def xload(hilo, xd, tsl):
                for cb in range(0, NCB, 4):
                    nc.sync.dma_start(
                        x4(hilo)[:, cb:cb + 4, tsl],
                        xd[cb * 128:(cb + 4) * 128, tsl].rearrange("(c p) t -> p c t", p=128))

            # Tiny first transfers: the first V matmul needs only wvhi/x
            # chunks 0-1, and the first-DMA pipeline latency (~2.2us of
            # DGE+sem overhead) gates the whole program start.
            nc.sync.dma_start(wv_(wvhi_sb, HD)[:, 0:2, :],
                              wvhi_d[0:256, :].rearrange("(c p) h -> p c h", p=128))
            nc.sync.dma_start(x4(0)[:, 0:2, 0:512],
                              xhi_d[0:256, 0:512].rearrange("(c p) t -> p c t", p=128))
            nc.sync.dma_start(wv_(wvhi_sb, HD)[:, 2:NCB, :],
                              wvhi_d[256:, :].rearrange("(c p) h -> p c h", p=128))
            nc.sync.dma_start(x4(0)[:, 2:4, 0:512],
                              xhi_d[256:512, 0:512].rearrange("(c p) t -> p c t", p=128))
            for cb in range(4, NCB, 4):
                nc.sync.dma_start(
                    x4(0)[:, cb:cb + 4, 0:512],
                    xhi_d[cb * 128:(cb + 4) * 128, 0:512].rearrange("(c p) t -> p c t", p=128))
            nc.sync.dma_start(wv_(wvlo_sb, HD), wvlo_d.rearrange("(c p) h -> p c h", p=128))
            xload(1, xlo_d, slice(0, 512))
            nc.sync.dma_start(wv_(wkhi_sb, HD), wkhi_d.rearrange("(c p) h -> p c h", p=128))
            nc.sync.dma_start(wv_(wqhi_sb, HPG * HD), wqhi_d.rearrange("(c p) m -> p c m", p=128))
            nc.sync.dma_start(cos_sb[:], cos_d)
            nc.sync.dma_start(sin_sb[:], sin_d)
            for tb in range(1, NTB):
                tsl = slice(tb * 512, (tb + 1) * 512)
                xload(0, xhi_d, tsl)
                xload(1, xlo_d, tsl)
                if tb == 1:
                    nc.sync.dma_start(maskE_sb[:], maskE_d)
                if tb == NTB - 1:
                    nc.sync.dma_start(wv_(wohi_sb, C), wohi_d.rearrange("(h p) c -> p h c", p=128))
                    nc.sync.dma_start(wv_(wolo_sb, C), wolo_d.rearrange("(h p) c -> p h c", p=128))

            TERMS_V = ((0, wvhi_sb), (1, wvhi_sb), (0, wvlo_sb))


            Exp = mybir.ActivationFunctionType.Exp

            with tc.tile_pool(name="attn_sb", bufs=6) as asb, \
                 tc.tile_pool(name="den_sb", bufs=2) as dsb, \
                 tc.tile_pool(name="yn_sb", bufs=4) as ysb, \
                 tc.tile_pool(name="out_sb", bufs=2) as osb, \
                 tc.tile_pool(name="rope_scr", bufs=4) as rsc:

                # ---------------- attention per query block ----------------
                # Blocks split into pairs sharing a [128,1024] PSUM tile:
                #   nwin=5: (lead, diag) + (m1, m2) + single m3
                #   nwin=4: (b0, diag) + (b1, b2)
                #   nwin=3: (b0, diag) + single b1
                #   nwin=2: (b0, diag)
                #   nwin=1: single diag (masked)
                def attn_scores(qb, stp, accp):
                    """Returns pms: list of (ap, jb) masked-prob tiles."""
                    nwin = min(qb, NWINB - 1) + 1
                    qsl = slice(qb * 512, (qb + 1) * 512)
                    j0 = qb - nwin + 1
                    lead = qb >= NWINB - 1    # first block is edge-masked
                    pms = []

                    def score_mm(dst, jb):
                        nc.tensor.matmul(dst, KT_sb[:, jb * 128:(jb + 1) * 128],
                                         QT4_sb[:, qsl], start=True, stop=True)

                    if nwin == 1:
                        stb = stp.tile([128, 1024], dt.float32, tag="stb", name="stbS1")
                        score_mm(stb[:, 0:512], qb)
                        pexp1 = asb.tile([128, 1024], dt.bfloat16, tag="pexpb", name="pexpS1", bufs=9)
                        nc.scalar.activation(pexp1[:, 0:512], stb[:, 0:512], Exp, scale=EXP_SCALE)
                        pm = asb.tile([128, 1024], dt.bfloat16, tag="pmask", bufs=3)
                        nc.vector.tensor_mul(pm[:, 0:512], pexp1[:, 0:512],
                                             maskE_sb[:, 512:1024])
                        pms.append((pm[:, 0:512], qb))
                        return pms, None

                    # edge pair: (first block, diag block) share one PSUM tile
                    # and one exp; when the first block is the window lead,
                    # ONE [128,1024] multiply applies both edge masks.
                    stb = stp.tile([128, 1024], dt.float32, tag="stb", name="stbE")
                    score_mm(stb[:, 0:512], j0)
                    score_mm(stb[:, 512:1024], qb)
                    pexp = asb.tile([128, 1024], dt.bfloat16, tag="pexpb", name="pexpE", bufs=9)
                    nc.scalar.activation(pexp[:], stb[:], Exp, scale=EXP_SCALE)
                    pm = asb.tile([128, 1024], dt.bfloat16, tag="pmask", bufs=3)
                    if lead:
                        nc.vector.tensor_mul(pm[:], pexp[:], maskE_sb[:])
                        pms.append((pm[:, 0:512], j0))
                        pms.append((pm[:, 512:1024], qb))
                    else:
                        nc.vector.tensor_mul(pm[:, 0:512], pexp[:, 512:1024],
                                             maskE_sb[:, 512:1024])
                        pms.append((pexp[:, 0:512], j0))
                        pms.append((pm[:, 0:512], qb))

                    # middle blocks: pairs then single.  The single-block
                    # matmul is returned as a deferred closure: it reuses the
                    # edge pair's PSUM slot, so emitting it immediately makes
                    # PE queue in-order behind that exp — the caller emits it
                    # after attn_rest's pumped Wo work instead.
                    mids = list(range(j0 + 1, qb))
                    while len(mids) >= 2:
                        a, b = mids[0], mids[1]
                        mids = mids[2:]
                        stb = stp.tile([128, 1024], dt.float32, tag="stb", name="stbM")
                        score_mm(stb[:, 0:512], a)
                        score_mm(stb[:, 512:1024], b)
                        pexpm = asb.tile([128, 1024], dt.bfloat16, tag="pexpb", name="pexpM", bufs=9)
                        nc.scalar.activation(pexpm[:], stb[:], Exp, scale=EXP_SCALE)
                        pms.append((pexpm[:, 0:512], a))
                        pms.append((pexpm[:, 512:1024], b))
                    s_emit = None
                    if mids:
                        a = mids[0]

                        def s_emit():
                            stb = stp.tile([128, 1024], dt.float32, tag="stb", name="stbS")
                            score_mm(stb[:, 0:512], a)
                            pexps = asb.tile([128, 1024], dt.bfloat16, tag="pexpb", name="pexpS", bufs=9)
                            nc.scalar.activation(pexps[:, 0:512], stb[:, 0:512], Exp, scale=EXP_SCALE)
                            pms.append((pexps[:, 0:512], a))
                    return pms, s_emit

                def wo_mms(wop, yhi, ylo, cb4s):
                    """Wo DoubleRow matmuls for the given output-column chunks.
                    All yhi-consuming matmuls emit before any ylo-consuming
                    one (across chunks too): ylo is the last-arriving input
                    (tail of Pool's queue), so PE slides 8 yhi matmuls ahead
                    of the wait instead of 4."""
                    y3 = lambda t: t[:].rearrange("p (k m) -> p k m", m=128)
                    tiles = [(cb4, wop.tile([128, 512], dt.float32, tag="wps", name="wps"))
                             for cb4 in cb4s]
                    for ys, ws, last in ((yhi, wohi_sb, False), (yhi, wolo_sb, False),
                                         (ylo, wohi_sb, True)):
                        first = ws is wohi_sb and ys is yhi
                        for cb4, wps in tiles:
                            csl = slice(cb4 * 512, (cb4 + 1) * 512)
                            for p2 in range(2):
                                nc.tensor.matmul(
                                    wps[:], y3(ys)[:, 2 * p2:2 * p2 + 2, :],
                                    wv_(ws, C)[:, 2 * p2:2 * p2 + 2, csl],
                                    start=(first and p2 == 0),
                                    stop=(last and p2 == 1), perf_mode=DRow)
                    return tiles

                def wo_finish(wo_qb, ostg, tiles):
                    """Evict Wo PSUM tiles (2 DVE + 2 ScalarE) + out DMA."""
                    rsl = slice(wo_qb * 128, (wo_qb + 1) * 128)
                    for cb4, wps in tiles:
                        csl = slice(cb4 * 512, (cb4 + 1) * 512)
                        if cb4 < 2:
                            nc.vector.tensor_copy(ostg[:, csl], wps[:])
                        else:
                            nc.scalar.copy(ostg[:, csl], wps[:])
                        if wo_qb >= NQB - 2:
                            nc.sync.dma_start(out_d[rsl, csl], ostg[:, csl])
                    if wo_qb < NQB - 2:
                        nc.sync.dma_start(out_d[rsl, :], ostg[:])

                def emit_wo(wop, wo_qb, yhi, ylo):
                    ostg = osb.tile([128, C], dt.bfloat16, tag="ostg", name="ostg")
                    tiles = wo_mms(wop, yhi, ylo, (0, 1, 2, 3))
                    wo_finish(wo_qb, ostg, tiles)

                def attn_rest(qb, pms, pend, npop, accp, wop, pump_units=True):
                    """PV + denominator + ynT fp8 pair, with Wo work pumped
                    between stages so PE always has ready work."""
                    nwin = len(pms)
                    units = []
                    if wop is not None:
                        for _ in range(min(npop, len(pend))):
                            wo_qb, yhi_p, ylo_p = pend.pop(0)
                            ostg_p = osb.tile([128, C], dt.bfloat16, tag="ostg", name="ostg")
                            tiles_p = []
                            units.append(("mm", wo_qb, yhi_p, ylo_p, ostg_p, tiles_p, (0, 1)))
                            units.append(("mm", wo_qb, yhi_p, ylo_p, ostg_p, tiles_p, (2, 3)))
                            units.append(("fin", wo_qb, yhi_p, ylo_p, ostg_p, tiles_p, None))
                    ui = [0]

                    def pump(n):
                        while n > 0 and ui[0] < len(units):
                            kind, wo_qb, yh, yl, ostg, tiles, cb4s = units[ui[0]]
                            if kind == "mm":
                                tiles += wo_mms(wop, yh, yl, cb4s)
                            else:
                                wo_finish(wo_qb, ostg, tiles)
                            ui[0] += 1
                            n -= 1

                    pump(1)
                    acc = accp.tile([128, 512], dt.float32, tag="acc", name="acc")
                    for i, (pap, jb) in enumerate(pms):
                        nc.tensor.matmul(acc[:], V_sb[:, jb * 128:(jb + 1) * 128], pap,
                                         start=(i == 0), stop=(i == nwin - 1))
                    pump(1)
                    # denominator tree: DVE pairwise adds, last add on GpSimd
                    work = [pap for pap, _ in pms]
                    while len(work) > 2:
                        nxt = []
                        for a, b in zip(work[0::2], work[1::2]):
                            t = asb.tile([128, 512], dt.bfloat16, tag="padd", name="padd", bufs=3)
                            nc.vector.tensor_add(t[:], a, b)
                            nxt.append(t[:])
                        if len(work) % 2:
                            nxt.append(work[-1])
                        work = nxt
                    if len(work) == 2:
                        t = dsb.tile([128, 512], dt.bfloat16, tag="psum_last")
                        nc.gpsimd.tensor_add(t[:], work[0], work[1])
                        work = [t[:]]
                    sbc = dsb.tile([128, 512], dt.float32, tag="sbc")
                    nc.gpsimd.partition_all_reduce(sbc[:], work[0], channels=128,
                                                   reduce_op=bass_isa.ReduceOp.add)
                    rbc = dsb.tile([128, 512], dt.bfloat16, tag="rbc")
                    with nc.allow_low_precision("softmax denominator reciprocal; 2e-2 rel-err budget"):
                        nc.vector.reciprocal(rbc[:], sbc[:])
                    t32 = ysb.tile([128, 512], dt.float32, tag="t32", name="t32")
                    nc.vector.tensor_mul(t32[:], acc[:], rbc[:])
                    yhi = ysb.tile([128, 512], f8, tag="yhi", name="yhi")
                    nc.vector.tensor_copy(yhi[:], t32[:])
                    ylo = ysb.tile([128, 512], f8, tag="ylo", name="ylo")
                    with nc.allow_low_precision("fp8 lo residual of ynT pair"):
                        nc.gpsimd.tensor_sub(ylo[:], t32[:], yhi[:])
                    if units:
                        pump(len(units))
                    pend.append((qb, yhi, ylo))

                # rope eviction: ScalarE PSUM->bf16 copy, 3 bf16 DVE muls,
                # GpSimd add into the destination (possibly strided).
                def rope_evict(ps, dst, tb, dst3=None):
                    # ScalarE evicts PSUM->bf16, then three all-SBUF bf16 DVE
                    # muls (2x mode).  The rotate-half swap rides on the
                    # OUTPUT AP base (outputs may cross bases; only SBUF
                    # input pairs must share a base — the sin table's row
                    # halves are pre-swapped on the host to keep them equal).
                    sl = slice(tb * 512, (tb + 1) * 512)
                    zb = rsc.tile([128, 512], dt.bfloat16, tag="zb", name="zb")
                    nc.scalar.copy(zb[:], ps[:])
                    t1 = rsc.tile([128, 512], dt.bfloat16, tag="t1")
                    t2 = rsc.tile([128, 512], dt.bfloat16, tag="t2")
                    nc.vector.tensor_mul(t1[:], zb[:], cos_sb[:, sl])
                    nc.vector.tensor_mul(t2[64:128, :], zb[0:64, :], sin_sb[0:64, sl])
                    nc.vector.tensor_mul(t2[0:64, :], zb[64:128, :], sin_sb[64:128, sl])
                    if dst3 is None:
                        nc.gpsimd.tensor_add(dst, t1[:], t2[:])
                    else:
                        r3 = lambda a: a.rearrange("p (a b) -> p a b", b=128)
                        nc.gpsimd.tensor_add(dst3, r3(t1[:]), r3(t2[:]))

                pend = []
                # ---------------- phase A: projections + attention qb0-2 ----
                with tc.tile_pool(name="proj_ps", bufs=3, space="PSUM") as pps, \
                     tc.tile_pool(name="v_ps", bufs=2, space="PSUM") as vpp, \
                     tc.tile_pool(name="stA_ps", bufs=1, space="PSUM") as stA, \
                     tc.tile_pool(name="accA_ps", bufs=1, space="PSUM") as accA:

                    def projV(tb):
                        for j in range(4):
                            t0 = (tb * 4 + j) * 128
                            vps = vpp.tile([128, 128], dt.float32, tag="vps", name="vps")
                            k = 0
                            for hilo, ws in TERMS_V:
                                for cb in range(0, NCB, 2):
                                    nc.tensor.matmul(
                                        vps[:], x4(hilo)[:, cb:cb + 2, t0:t0 + 128],
                                        wv_(ws, HD)[:, cb:cb + 2, :],
                                        start=(k == 0), stop=(k == 23), perf_mode=DRow)
                                    k += 1
                            nc.scalar.copy(V_sb[:, t0:t0 + 128], vps[:])

                    def projKQ(tb):
                        tsl = slice(tb * 512, (tb + 1) * 512)
                        # K 2-term A-variant: (xhi+xlo) @ Wkhi via stride-0
                        # broadcast weight chunks (full x precision, no Wklo)
                        x2v = x2_sb[:].rearrange("p (c h t) -> p c h t", h=2, t=t_len)
                        kps = pps.tile([128, 512], dt.float32, tag="ps", name="kps")
                        for cb in range(NCB):
                            wb = wv_(wkhi_sb, HD)[:, cb:cb + 1, :].broadcast_to([128, 2, HD])
                            nc.tensor.matmul(
                                kps[:], wb, x2v[:, cb, :, tsl],
                                start=(cb == 0), stop=(cb == NCB - 1), perf_mode=DRow)
                        rope_evict(kps, KT_sb[:, tsl], tb)
                        # Q 2-term A-variant: (xhi+xlo) @ Wqhi, same
                        # stride-0 broadcast scheme as K above.
                        for h in range(HPG):
                            qps = pps.tile([128, 512], dt.float32, tag="ps", name="qps")
                            for cb in range(NCB):
                                wb = wv_(wqhi_sb, HPG * HD)[:, cb:cb + 1, h * HD:(h + 1) * HD] \
                                    .broadcast_to([128, 2, HD])
                                nc.tensor.matmul(
                                    qps[:], wb, x2v[:, cb, :, tsl],
                                    start=(cb == 0), stop=(cb == NCB - 1), perf_mode=DRow)
                            dst3 = QT4_sb[:].rearrange("p (q s) -> p q s", s=HPG * 128)[
                                :, 4 * tb:4 * tb + 4, h * 128:(h + 1) * 128]
                            rope_evict(qps, None, tb, dst3=dst3)

                    # Software-pipelined emission: iteration tb emits this tb's
                    # projections, then scores(tb-1), then attn_rest(tb-2) —
                    # every exp/mask/denominator chain gets a full iteration
                    # of slack before its consumer, so PE never queues behind
                    # a not-yet-ready chain.
                    prev = None
                    for tb in range(NTB):
                        projV(tb)
                        projKQ(tb)
                        if tb >= 1:
                            if prev is not None:
                                attn_rest(prev[0], prev[1], pend, 0, accA, None)
                            pmsA, sA = attn_scores(tb - 1, stA, accA)
                            if sA:
                                sA()
                            prev = (tb - 1, pmsA)

                # ---------------- phase B: qb15 first, then qb3..14 ----------
                # (pipelined: iteration k emits scores(qb_k) then
                # attn_rest(qb_{k-1}); `prev` carries qb2 over from phase A)
                with tc.tile_pool(name="st_ps", bufs=2, space="PSUM") as stp, \
                     tc.tile_pool(name="acc_ps", bufs=2, space="PSUM") as accp, \
                     tc.tile_pool(name="wo_ps", bufs=2, space="PSUM") as wop:

                    # 2-deep pipeline: scores(qb_k) emits with rest(qb_{k-2}),
                    # so the exp/mask/denominator chain gets ~2 iterations of
                    # slack before PV consumes it.
                    order = [NQB - 1] + list(range(3, NQB - 1))
                    queue = [prev]
                    for qb in order:
                        pms, s = attn_scores(qb, stp, accp)
                        queue.append((qb, pms))
                        if s:
                            s()
                        if len(queue) > 2:
                            npop = 2 if len(pend) > 1 else 1
                            attn_rest(*queue.pop(0), pend, npop, accp, wop)

                    for ent in queue:
                        attn_rest(*ent, pend, 2, accp, wop)
                    while pend:
                        wo_qb, yhi_p, ylo_p = pend.pop(0)
                        emit_wo(wop, wo_qb, yhi_p, ylo_p)

    nc.compile()
    return nc


def _get_nc(t_len=T):
    if t_len not in _NC_CACHE:
        _NC_CACHE[t_len] = build_nc(t_len)
    return _NC_CACHE[t_len]


def host_inputs(x, Wq, Wk, Wv, Wo, t_len=T):
    """Per-core input shards (8 dicts)."""
    x = np.asarray(x, np.float32)
    Wq = np.asarray(Wq, np.float32) * WSC
    Wk = np.asarray(Wk, np.float32) * WSC
    Wv = np.asarray(Wv, np.float32) * WSC
    Wo = np.asarray(Wo, np.float32) * WSC
    cosT, sin_swap = _rope_tables(t_len)
    common = {
        "cosT": cosT.astype(BF16),
        "sinT": sin_swap.astype(BF16),
        "maskE": _mask_edge4().astype(BF16),
    }
    in_maps = []
    for core in range(NCORES):
        b, hg = core // TPG, core % TPG
        m = dict(common)
        m["xhi"], m["xlo"] = _split_f8(np.ascontiguousarray(x[b, :t_len, :].T))
        m["wqhi"] = _split_f8(Wq[:, hg * HPG * HD:(hg + 1) * HPG * HD])[0]
        m["wkhi"] = _split_f8(Wk[:, hg * HD:(hg + 1) * HD])[0]
        m["wvhi"], m["wvlo"] = _split_f8(Wv[:, hg * HD:(hg + 1) * HD])
        m["wohi"], m["wolo"] = _split_f8(Wo[hg * HPG * HD:(hg + 1) * HPG * HD, :])
        in_maps.append(m)
    return in_maps


def kernel(x, Wq, Wk, Wv, Wo):
    from concourse import bass_utils

    nc = _get_nc(T)
    in_maps = host_inputs(x, Wq, Wk, Wv, Wo, T)
    res = bass_utils.run_bass_kernel_spmd(nc, in_maps, core_ids=list(range(NCORES)))
    out = np.zeros((B, T, C), np.float32)
    for core in range(NCORES):
        out[core // TPG] += res.results[core]["out"].astype(np.float32)
    out *= 1.0 / (WSC * WSC)
    return out


def core_reference(x_b, Wq, Wk, Wv, Wo, hg, t_len=T):
    """Numpy reference of one core's partial output (f64 math, for dev tests)."""
    xb = np.asarray(x_b, np.float64)[:t_len]
    q = xb @ np.float64(Wq[:, hg * HPG * HD:(hg + 1) * HPG * HD])    # [T, 512]
    k = xb @ np.float64(Wk[:, hg * HD:(hg + 1) * HD])                # [T, 128]
    v = xb @ np.float64(Wv[:, hg * HD:(hg + 1) * HD])
    inv = (1.0 / (np.float32(ROPE_BASE) ** (np.arange(0, HD, 2, dtype=np.float32) / np.float32(HD)))).astype(np.float32)
    ang = np.arange(t_len, dtype=np.float32)[:, None] * inv[None, :]
    cos = np.concatenate([np.cos(ang), np.cos(ang)], axis=1).astype(np.float64)
    sin = np.concatenate([np.sin(ang), np.sin(ang)], axis=1).astype(np.float64)

    def rope(z):
        zsw = np.concatenate([-z[:, HD // 2:], z[:, :HD // 2]], axis=1)
        return z * cos + zsw * sin

    out = np.zeros((t_len, C), np.float64)
    i = np.arange(t_len)[:, None]
    j = np.arange(t_len)[None, :]
    allowed = (j <= i) & (i - j < WIN)
    kr = rope(k)
    for h in range(HPG):
        qh = rope(q[:, h * HD:(h + 1) * HD]) * (1.0 / np.sqrt(np.float64(HD)))
        s = qh @ kr.T
        s = np.where(allowed, s, -np.inf)
        p = np.exp(s - s.max(axis=1, keepdims=True))
        p /= p.sum(axis=1, keepdims=True)
        y = p @ v
        out += y @ np.float64(Wo[hg * HPG * HD + h * HD: hg * HPG * HD + (h + 1) * HD, :])
    return out.astype(np.float32)
